# revision 17
# baseline (speedup 1.0000x reference)
"""LIF spike recurrence kernel for Trainium2 (8 NeuronCores, SPMD). v12.

Problem: x [32, 128, 32, 32, 8] f32, recurrence over last (time) dim:
    u_t = TAU * u_{t-1} * (1 - o_{t-1}) + x_t
    o_t = 1[u_t - VTH > 0]
Output: o [32, 128, 32, 32, 8] f32 (0.0 / 1.0 spikes).

Design (all facts hardware-probed):
  - Shard batch (32) across 8 cores -> 4/core; host pre-transposes each shard
    to plane-major [P=128, T=8, NPP=4096] so every SBUF access is contiguous
    (strided fp32 STT costs ~1.6x; contiguous runs at (FD+151)/0.96 ns exact,
    back-to-back with ~40ns gaps).
  - One mega-tile [P, T, NPP] per core; input DMA chunked (planes 0/1 in
    interleaved column chunks, so step-1 compute starts ~4 us earlier).
  - Exact fp32 recurrence on DVE, bit-identical to the reference:
       c   = (u_{t-1} <= VTH) * u_{t-1}     (STT is_le/mult; x{0,1} exact)
       u_t = c * TAU + x_t   in place       (TAU=2^-2 exact; single round)
  - Spike via ScalarE: o8_t = Sign(u_t - VTH) -> int8 {-1,0,1}, one ACTIVATE
    per plane ((FD+352)/1.2 ns, no bubble), fully hidden under DVE. Host maps
    >0 to 1.0f (exact). int8 output cuts out-DMA 4x vs f32.
  - Output DMA rides the Scalar HWDGE ring (input uses Sync's) so the queues
    never serialize.
  - v12 over v6: the DVE chain (62us serial) outruns the one-ring input
    stream only at the start; early steps stalled ~2-3us waiting for plane
    arrivals, and step 1 started at ~15us. Now planes 0/1 interleave in
    1K-column chunks (step 1 starts ~2.6us earlier) and the second halves
    of planes 2-4 ride the Scalar HWDGE ring (enqueued before any output
    DMA, so they clear the ring first), which pulls plane arrivals ahead of
    the DVE chain for the whole run.
  - Step 1 is column-chunked and step 7 column-quartered (signs/outs
    interleaved) to shorten the pipeline head and tail.
"""

import numpy as np

TAU = 0.25
VTH = 0.3
N_CORES = 8
P = 128
T = 8
B_LOC = 4  # batches per core
PIX_PER_CORE = B_LOC * 128 * 32 * 32  # 524288
NPP = PIX_PER_CORE // P  # 4096 pixels per partition

_CACHE = {}

# Config key (A/B-tested on hardware):
#   h<k>  : planes 0/1 interleave chunk size k
#   sr<n> : second halves of planes 2..n+1 ride the Scalar ring
#   sp24  : planes 2 and 4 ride the Scalar ring whole (enqueued before any
#           output DMA), pulling early-plane arrivals ahead of the DVE chain
#   t8    : split the last step-7 quarter into two 512-col chunks
CFG = "v13_h2048_sr0_sp24_t8"


def _parse(key):
    h, sr = 1024, 3
    for tok in key.split("_"):
        if tok.startswith("h") and tok[1:].isdigit():
            h = int(tok[1:])
        if tok.startswith("sr") and tok[2:].isdigit():
            sr = int(tok[2:])
    return dict(
        h=h, sr=sr, noaeb="noaeb" in key,
        sp24="sp24" in key, t8="t8" in key,
    )


def _build_nc(key=None):
    if key is None:
        key = CFG
    cfg = _parse(key)
    import concourse.tile as tile
    from concourse import bacc, mybir

    f32 = mybir.dt.float32
    i8 = mybir.dt.int8
    Alu = mybir.AluOpType
    AF = mybir.ActivationFunctionType

    nc = bacc.Bacc(
        "TRN2",
        target_bir_lowering=False,
        debug=False,
        enable_asserts=False,
        num_devices=N_CORES,
    )
    x_d = nc.dram_tensor("x", [P, T, NPP], f32, kind="ExternalInput").ap()
    o_d = nc.dram_tensor("o", [P, T, NPP], i8, kind="ExternalOutput").ap()

    # ACT activation bias needs a pre-registered const AP.
    cb = nc.alloc_sbuf_tensor("const-f32-negvth", [128, 1], f32)
    nc.gpsimd.memset(cb.ap(), -VTH)
    nc.const_aps.aps[(f32, -VTH)] = cb.ap()
    if cfg["noaeb"]:
        nc.multi_engine_barrier(
            [mybir.EngineType.Pool, mybir.EngineType.Activation]
        )
    else:
        nc.all_engine_barrier()

    H = cfg["h"]
    SR = cfg["sr"]  # planes 2..SR+1 second halves on scalar ring

    with tile.TileContext(nc) as tc:
        with tc.tile_pool(name="pp", bufs=1) as pp:
            xt = pp.tile([P, T, NPP], f32, tag="xt")
            c = pp.tile([P, NPP], f32, tag="c")
            o8 = pp.tile([P, T, NPP], i8, tag="o8")

            # Scalar-ring planes first: their enqueues precede every output
            # DMA on that ring, so they stream concurrently with Sync's.
            sp = {2, 4} if cfg["sp24"] else set()
            for t in sorted(sp):
                nc.scalar.dma_start(xt[:, t, :], x_d[:, t, :])
            # Input: planes 0/1 interleaved in H-column chunks on Sync.
            bounds = list(range(0, NPP, H)) + [NPP]
            chunks = list(zip(bounds[:-1], bounds[1:]))
            for lo, hi in chunks:
                nc.sync.dma_start(xt[:, 0, lo:hi], x_d[:, 0, lo:hi])
                nc.sync.dma_start(xt[:, 1, lo:hi], x_d[:, 1, lo:hi])
            # Planes 2..SR+1: first half Sync, second half Scalar ring
            # (enqueued before any output DMA so they clear the ring first).
            for t in range(2, T):
                if t in sp:
                    continue
                if 2 <= t < 2 + SR:
                    nc.sync.dma_start(xt[:, t, :2048], x_d[:, t, :2048])
                    nc.scalar.dma_start(xt[:, t, 2048:], x_d[:, t, 2048:])
                else:
                    nc.sync.dma_start(xt[:, t, :], x_d[:, t, :])

            def cu(t, sl):
                up = xt[:, t - 1, sl]
                nc.vector.scalar_tensor_tensor(
                    c[:, sl], up, VTH, up, op0=Alu.is_le, op1=Alu.mult
                )
                nc.vector.scalar_tensor_tensor(
                    xt[:, t, sl], c[:, sl], TAU, xt[:, t, sl],
                    op0=Alu.mult, op1=Alu.add,
                )

            # Plane 0: u_0 = x_0, spike immediately.
            nc.scalar.activation(o8[:, 0, :], xt[:, 0, :], AF.Sign, bias=-VTH)
            nc.scalar.dma_start(o_d[:, 0, :], o8[:, 0, :])

            # Step 1: chunked (chases the chunked DMAs).
            for lo, hi in chunks:
                cu(1, slice(lo, hi))
            nc.scalar.activation(o8[:, 1, :], xt[:, 1, :], AF.Sign, bias=-VTH)
            nc.scalar.dma_start(o_d[:, 1, :], o8[:, 1, :])

            # Steps 2..T-2: full-plane ops (minimal op count).
            for t in range(2, T - 1):
                cu(t, slice(0, NPP))
                nc.scalar.activation(o8[:, t, :], xt[:, t, :], AF.Sign, bias=-VTH)
                nc.scalar.dma_start(o_d[:, t, :], o8[:, t, :])

            # Step T-1: chunked with interleaved signs/outs (short tail); the
            # final chunks spike on the (now otherwise-done) DVE via is_gt ->
            # int8 {0,1} (host maps >0 so both encodings work).
            t7b = [0, 1024, 2048, 3072, NPP]
            if cfg["t8"]:
                t7b = [0, 1024, 2048, 3072, 3584, NPP]
            t7chunks = list(zip(t7b[:-1], t7b[1:]))
            for q, (lo, hi) in enumerate(t7chunks):
                sl = slice(lo, hi)
                cu(T - 1, sl)
                if q >= len(t7chunks) - (2 if cfg["t8"] else 1):
                    nc.vector.tensor_scalar(
                        o8[:, T - 1, sl], xt[:, T - 1, sl], VTH, None,
                        op0=Alu.is_gt,
                    )
                else:
                    nc.scalar.activation(
                        o8[:, T - 1, sl], xt[:, T - 1, sl], AF.Sign, bias=-VTH
                    )
                nc.scalar.dma_start(o_d[:, T - 1, sl], o8[:, T - 1, sl])
    nc.compile()
    return nc


def _get_nc(key=None):
    k = key or CFG
    if k not in _CACHE:
        _CACHE[k] = _build_nc(k)
    return _CACHE[k]


def _shard(x: np.ndarray):
    xs = np.ascontiguousarray(x, dtype=np.float32)
    return [
        np.ascontiguousarray(
            xs[i * B_LOC : (i + 1) * B_LOC].reshape(P, NPP, T).transpose(0, 2, 1)
        )
        for i in range(N_CORES)
    ]


def _run(in_maps, key=None, **kwargs):
    from concourse.bass_utils import run_bass_kernel_spmd

    nc = _get_nc(key)
    return run_bass_kernel_spmd(nc, in_maps, core_ids=list(range(N_CORES)), **kwargs)


def kernel(x: np.ndarray) -> np.ndarray:
    in_maps = [{"x": s} for s in _shard(x)]
    res = _run(in_maps)
    outs = []
    for i in range(N_CORES):
        s8 = res.results[i]["o"]  # [P, T, NPP] int8 sign values
        o = (s8 > 0).transpose(0, 2, 1).astype(np.float32)  # [P, NPP, T]
        outs.append(o.reshape(B_LOC, 128, 32, 32, T))
    return np.concatenate(outs, axis=0)


# revision 18
# speedup vs baseline: 1.1200x; 1.1200x over previous
"""LIF spike recurrence kernel for Trainium2 (8 NeuronCores, SPMD). v12.

Problem: x [32, 128, 32, 32, 8] f32, recurrence over last (time) dim:
    u_t = TAU * u_{t-1} * (1 - o_{t-1}) + x_t
    o_t = 1[u_t - VTH > 0]
Output: o [32, 128, 32, 32, 8] f32 (0.0 / 1.0 spikes).

Design (all facts hardware-probed):
  - Shard batch (32) across 8 cores -> 4/core; host pre-transposes each shard
    to plane-major [P=128, T=8, NPP=4096] so every SBUF access is contiguous
    (strided fp32 STT costs ~1.6x; contiguous runs at (FD+151)/0.96 ns exact,
    back-to-back with ~40ns gaps).
  - One mega-tile [P, T, NPP] per core; input DMA chunked (planes 0/1 in
    interleaved column chunks, so step-1 compute starts ~4 us earlier).
  - Exact fp32 recurrence on DVE, bit-identical to the reference:
       c   = (u_{t-1} <= VTH) * u_{t-1}     (STT is_le/mult; x{0,1} exact)
       u_t = c * TAU + x_t   in place       (TAU=2^-2 exact; single round)
  - Spike via ScalarE: o8_t = Sign(u_t - VTH) -> int8 {-1,0,1}, one ACTIVATE
    per plane ((FD+352)/1.2 ns, no bubble), fully hidden under DVE. Host maps
    >0 to 1.0f (exact). int8 output cuts out-DMA 4x vs f32.
  - Output DMA rides the Scalar HWDGE ring (input uses Sync's) so the queues
    never serialize.
  - v12 over v6: the DVE chain (62us serial) outruns the one-ring input
    stream only at the start; early steps stalled ~2-3us waiting for plane
    arrivals, and step 1 started at ~15us. Now planes 0/1 interleave in
    1K-column chunks (step 1 starts ~2.6us earlier) and the second halves
    of planes 2-4 ride the Scalar HWDGE ring (enqueued before any output
    DMA, so they clear the ring first), which pulls plane arrivals ahead of
    the DVE chain for the whole run.
  - Step 1 is column-chunked and step 7 column-quartered (signs/outs
    interleaved) to shorten the pipeline head and tail.
"""

import numpy as np

TAU = 0.25
VTH = 0.3
N_CORES = 8
P = 128
T = 8
B_LOC = 4  # batches per core
PIX_PER_CORE = B_LOC * 128 * 32 * 32  # 524288
NPP = PIX_PER_CORE // P  # 4096 pixels per partition

_CACHE = {}

# Config key (A/B-tested on hardware):
#   h<k>  : planes 0/1 interleave chunk size k
#   sr<n> : second halves of planes 2..n+1 ride the Scalar ring
#   sp24  : planes 2 and 4 ride the Scalar ring whole (enqueued before any
#           output DMA), pulling early-plane arrivals ahead of the DVE chain
#   t8    : split the last step-7 quarter into two 512-col chunks
CFG = "v13_h2048_sr0_t8"


def _parse(key):
    h, sr = 1024, 3
    for tok in key.split("_"):
        if tok.startswith("h") and tok[1:].isdigit():
            h = int(tok[1:])
        if tok.startswith("sr") and tok[2:].isdigit():
            sr = int(tok[2:])
    return dict(
        h=h, sr=sr, noaeb="noaeb" in key,
        sp24="sp24" in key, t8="t8" in key,
    )


def _build_nc(key=None):
    if key is None:
        key = CFG
    cfg = _parse(key)
    import concourse.tile as tile
    from concourse import bacc, mybir

    f32 = mybir.dt.float32
    i8 = mybir.dt.int8
    Alu = mybir.AluOpType
    AF = mybir.ActivationFunctionType

    nc = bacc.Bacc(
        "TRN2",
        target_bir_lowering=False,
        debug=False,
        enable_asserts=False,
        num_devices=N_CORES,
    )
    x_d = nc.dram_tensor("x", [P, T, NPP], f32, kind="ExternalInput").ap()
    o_d = nc.dram_tensor("o", [P, T, NPP], i8, kind="ExternalOutput").ap()

    # ACT activation bias needs a pre-registered const AP.
    cb = nc.alloc_sbuf_tensor("const-f32-negvth", [128, 1], f32)
    nc.gpsimd.memset(cb.ap(), -VTH)
    nc.const_aps.aps[(f32, -VTH)] = cb.ap()
    if cfg["noaeb"]:
        nc.multi_engine_barrier(
            [mybir.EngineType.Pool, mybir.EngineType.Activation]
        )
    else:
        nc.all_engine_barrier()

    H = cfg["h"]
    SR = cfg["sr"]  # planes 2..SR+1 second halves on scalar ring

    with tile.TileContext(nc) as tc:
        with tc.tile_pool(name="pp", bufs=1) as pp:
            xt = pp.tile([P, T, NPP], f32, tag="xt")
            c = pp.tile([P, NPP], f32, tag="c")
            o8 = pp.tile([P, T, NPP], i8, tag="o8")

            # Scalar-ring planes first: their enqueues precede every output
            # DMA on that ring, so they stream concurrently with Sync's.
            sp = {2, 4} if cfg["sp24"] else set()
            for t in sorted(sp):
                nc.scalar.dma_start(xt[:, t, :], x_d[:, t, :])
            # Input: planes 0/1 interleaved in H-column chunks on Sync.
            bounds = list(range(0, NPP, H)) + [NPP]
            chunks = list(zip(bounds[:-1], bounds[1:]))
            for lo, hi in chunks:
                nc.sync.dma_start(xt[:, 0, lo:hi], x_d[:, 0, lo:hi])
                nc.sync.dma_start(xt[:, 1, lo:hi], x_d[:, 1, lo:hi])
            # Planes 2..SR+1: first half Sync, second half Scalar ring
            # (enqueued before any output DMA so they clear the ring first).
            for t in range(2, T):
                if t in sp:
                    continue
                if 2 <= t < 2 + SR:
                    nc.sync.dma_start(xt[:, t, :2048], x_d[:, t, :2048])
                    nc.scalar.dma_start(xt[:, t, 2048:], x_d[:, t, 2048:])
                else:
                    nc.sync.dma_start(xt[:, t, :], x_d[:, t, :])

            def cu(t, sl):
                up = xt[:, t - 1, sl]
                nc.vector.scalar_tensor_tensor(
                    c[:, sl], up, VTH, up, op0=Alu.is_le, op1=Alu.mult
                )
                nc.vector.scalar_tensor_tensor(
                    xt[:, t, sl], c[:, sl], TAU, xt[:, t, sl],
                    op0=Alu.mult, op1=Alu.add,
                )

            # Plane 0: u_0 = x_0, spike immediately.
            nc.scalar.activation(o8[:, 0, :], xt[:, 0, :], AF.Sign, bias=-VTH)
            nc.scalar.dma_start(o_d[:, 0, :], o8[:, 0, :])

            # Step 1: chunked (chases the chunked DMAs).
            for lo, hi in chunks:
                cu(1, slice(lo, hi))
            nc.scalar.activation(o8[:, 1, :], xt[:, 1, :], AF.Sign, bias=-VTH)
            nc.scalar.dma_start(o_d[:, 1, :], o8[:, 1, :])

            # Steps 2..T-2: full-plane ops (minimal op count).
            for t in range(2, T - 1):
                cu(t, slice(0, NPP))
                nc.scalar.activation(o8[:, t, :], xt[:, t, :], AF.Sign, bias=-VTH)
                nc.scalar.dma_start(o_d[:, t, :], o8[:, t, :])

            # Step T-1: chunked with interleaved signs/outs (short tail); the
            # final chunks spike on the (now otherwise-done) DVE via is_gt ->
            # int8 {0,1} (host maps >0 so both encodings work).
            t7b = [0, 1024, 2048, 3072, NPP]
            if cfg["t8"]:
                t7b = [0, 1024, 2048, 3072, 3584, NPP]
            t7chunks = list(zip(t7b[:-1], t7b[1:]))
            for q, (lo, hi) in enumerate(t7chunks):
                sl = slice(lo, hi)
                cu(T - 1, sl)
                if q >= len(t7chunks) - (2 if cfg["t8"] else 1):
                    nc.vector.tensor_scalar(
                        o8[:, T - 1, sl], xt[:, T - 1, sl], VTH, None,
                        op0=Alu.is_gt,
                    )
                else:
                    nc.scalar.activation(
                        o8[:, T - 1, sl], xt[:, T - 1, sl], AF.Sign, bias=-VTH
                    )
                nc.scalar.dma_start(o_d[:, T - 1, sl], o8[:, T - 1, sl])
    nc.compile()
    return nc


def _get_nc(key=None):
    k = key or CFG
    if k not in _CACHE:
        _CACHE[k] = _build_nc(k)
    return _CACHE[k]


def _shard(x: np.ndarray):
    xs = np.ascontiguousarray(x, dtype=np.float32)
    return [
        np.ascontiguousarray(
            xs[i * B_LOC : (i + 1) * B_LOC].reshape(P, NPP, T).transpose(0, 2, 1)
        )
        for i in range(N_CORES)
    ]


def _run(in_maps, key=None, **kwargs):
    from concourse.bass_utils import run_bass_kernel_spmd

    nc = _get_nc(key)
    return run_bass_kernel_spmd(nc, in_maps, core_ids=list(range(N_CORES)), **kwargs)


def kernel(x: np.ndarray) -> np.ndarray:
    in_maps = [{"x": s} for s in _shard(x)]
    res = _run(in_maps)
    outs = []
    for i in range(N_CORES):
        s8 = res.results[i]["o"]  # [P, T, NPP] int8 sign values
        o = (s8 > 0).transpose(0, 2, 1).astype(np.float32)  # [P, NPP, T]
        outs.append(o.reshape(B_LOC, 128, 32, 32, T))
    return np.concatenate(outs, axis=0)


# revision 19
# speedup vs baseline: 1.1458x; 1.0230x over previous
"""LIF spike recurrence kernel for Trainium2 (8 NeuronCores, SPMD). v12.

Problem: x [32, 128, 32, 32, 8] f32, recurrence over last (time) dim:
    u_t = TAU * u_{t-1} * (1 - o_{t-1}) + x_t
    o_t = 1[u_t - VTH > 0]
Output: o [32, 128, 32, 32, 8] f32 (0.0 / 1.0 spikes).

Design (all facts hardware-probed):
  - Shard batch (32) across 8 cores -> 4/core; host pre-transposes each shard
    to plane-major [P=128, T=8, NPP=4096] so every SBUF access is contiguous
    (strided fp32 STT costs ~1.6x; contiguous runs at (FD+151)/0.96 ns exact,
    back-to-back with ~40ns gaps).
  - One mega-tile [P, T, NPP] per core; input DMA chunked (planes 0/1 in
    interleaved column chunks, so step-1 compute starts ~4 us earlier).
  - Exact fp32 recurrence on DVE, bit-identical to the reference:
       c   = (u_{t-1} <= VTH) * u_{t-1}     (STT is_le/mult; x{0,1} exact)
       u_t = c * TAU + x_t   in place       (TAU=2^-2 exact; single round)
  - Spike via ScalarE: o8_t = Sign(u_t - VTH) -> int8 {-1,0,1}, one ACTIVATE
    per plane ((FD+352)/1.2 ns, no bubble), fully hidden under DVE. Host maps
    >0 to 1.0f (exact). int8 output cuts out-DMA 4x vs f32.
  - Output DMA rides the Scalar HWDGE ring (input uses Sync's) so the queues
    never serialize.
  - v12 over v6: the DVE chain (62us serial) outruns the one-ring input
    stream only at the start; early steps stalled ~2-3us waiting for plane
    arrivals, and step 1 started at ~15us. Now planes 0/1 interleave in
    1K-column chunks (step 1 starts ~2.6us earlier) and the second halves
    of planes 2-4 ride the Scalar HWDGE ring (enqueued before any output
    DMA, so they clear the ring first), which pulls plane arrivals ahead of
    the DVE chain for the whole run.
  - Step 1 is column-chunked and step 7 column-quartered (signs/outs
    interleaved) to shorten the pipeline head and tail.
"""

import numpy as np

TAU = 0.25
VTH = 0.3
N_CORES = 8
P = 128
T = 8
B_LOC = 4  # batches per core
PIX_PER_CORE = B_LOC * 128 * 32 * 32  # 524288
NPP = PIX_PER_CORE // P  # 4096 pixels per partition

_CACHE = {}

# Config key (A/B-tested on hardware):
#   h<k>  : planes 0/1 interleave chunk size k
#   sr<n> : second halves of planes 2..n+1 ride the Scalar ring
#   sp24  : planes 2 and 4 ride the Scalar ring whole (enqueued before any
#           output DMA), pulling early-plane arrivals ahead of the DVE chain
#   t8    : split the last step-7 quarter into two 512-col chunks
CFG = "v12_h2048_sr0"


def _parse(key):
    h, sr = 1024, 3
    for tok in key.split("_"):
        if tok.startswith("h") and tok[1:].isdigit():
            h = int(tok[1:])
        if tok.startswith("sr") and tok[2:].isdigit():
            sr = int(tok[2:])
    return dict(
        h=h, sr=sr, noaeb="noaeb" in key,
        sp24="sp24" in key, t8="t8" in key,
    )


def _build_nc(key=None):
    if key is None:
        key = CFG
    cfg = _parse(key)
    import concourse.tile as tile
    from concourse import bacc, mybir

    f32 = mybir.dt.float32
    i8 = mybir.dt.int8
    Alu = mybir.AluOpType
    AF = mybir.ActivationFunctionType

    nc = bacc.Bacc(
        "TRN2",
        target_bir_lowering=False,
        debug=False,
        enable_asserts=False,
        num_devices=N_CORES,
    )
    x_d = nc.dram_tensor("x", [P, T, NPP], f32, kind="ExternalInput").ap()
    o_d = nc.dram_tensor("o", [P, T, NPP], i8, kind="ExternalOutput").ap()

    # ACT activation bias needs a pre-registered const AP.
    cb = nc.alloc_sbuf_tensor("const-f32-negvth", [128, 1], f32)
    nc.gpsimd.memset(cb.ap(), -VTH)
    nc.const_aps.aps[(f32, -VTH)] = cb.ap()
    if cfg["noaeb"]:
        nc.multi_engine_barrier(
            [mybir.EngineType.Pool, mybir.EngineType.Activation]
        )
    else:
        nc.all_engine_barrier()

    H = cfg["h"]
    SR = cfg["sr"]  # planes 2..SR+1 second halves on scalar ring

    with tile.TileContext(nc) as tc:
        with tc.tile_pool(name="pp", bufs=1) as pp:
            xt = pp.tile([P, T, NPP], f32, tag="xt")
            c = pp.tile([P, NPP], f32, tag="c")
            o8 = pp.tile([P, T, NPP], i8, tag="o8")

            # Scalar-ring planes first: their enqueues precede every output
            # DMA on that ring, so they stream concurrently with Sync's.
            sp = {2, 4} if cfg["sp24"] else set()
            for t in sorted(sp):
                nc.scalar.dma_start(xt[:, t, :], x_d[:, t, :])
            # Input: planes 0/1 interleaved in H-column chunks on Sync.
            bounds = list(range(0, NPP, H)) + [NPP]
            chunks = list(zip(bounds[:-1], bounds[1:]))
            for lo, hi in chunks:
                nc.sync.dma_start(xt[:, 0, lo:hi], x_d[:, 0, lo:hi])
                nc.sync.dma_start(xt[:, 1, lo:hi], x_d[:, 1, lo:hi])
            # Planes 2..SR+1: first half Sync, second half Scalar ring
            # (enqueued before any output DMA so they clear the ring first).
            for t in range(2, T):
                if t in sp:
                    continue
                if 2 <= t < 2 + SR:
                    nc.sync.dma_start(xt[:, t, :2048], x_d[:, t, :2048])
                    nc.scalar.dma_start(xt[:, t, 2048:], x_d[:, t, 2048:])
                else:
                    nc.sync.dma_start(xt[:, t, :], x_d[:, t, :])

            def cu(t, sl):
                up = xt[:, t - 1, sl]
                nc.vector.scalar_tensor_tensor(
                    c[:, sl], up, VTH, up, op0=Alu.is_le, op1=Alu.mult
                )
                nc.vector.scalar_tensor_tensor(
                    xt[:, t, sl], c[:, sl], TAU, xt[:, t, sl],
                    op0=Alu.mult, op1=Alu.add,
                )

            # Plane 0: u_0 = x_0, spike immediately.
            nc.scalar.activation(o8[:, 0, :], xt[:, 0, :], AF.Sign, bias=-VTH)
            nc.scalar.dma_start(o_d[:, 0, :], o8[:, 0, :])

            # Step 1: chunked (chases the chunked DMAs).
            for lo, hi in chunks:
                cu(1, slice(lo, hi))
            nc.scalar.activation(o8[:, 1, :], xt[:, 1, :], AF.Sign, bias=-VTH)
            nc.scalar.dma_start(o_d[:, 1, :], o8[:, 1, :])

            # Steps 2..T-2: full-plane ops (minimal op count).
            for t in range(2, T - 1):
                cu(t, slice(0, NPP))
                nc.scalar.activation(o8[:, t, :], xt[:, t, :], AF.Sign, bias=-VTH)
                nc.scalar.dma_start(o_d[:, t, :], o8[:, t, :])

            # Step T-1: chunked with interleaved signs/outs (short tail); the
            # final chunks spike on the (now otherwise-done) DVE via is_gt ->
            # int8 {0,1} (host maps >0 so both encodings work).
            t7b = [0, 1024, 2048, 3072, NPP]
            if cfg["t8"]:
                t7b = [0, 1024, 2048, 3072, 3584, NPP]
            t7chunks = list(zip(t7b[:-1], t7b[1:]))
            for q, (lo, hi) in enumerate(t7chunks):
                sl = slice(lo, hi)
                cu(T - 1, sl)
                if q >= len(t7chunks) - (2 if cfg["t8"] else 1):
                    nc.vector.tensor_scalar(
                        o8[:, T - 1, sl], xt[:, T - 1, sl], VTH, None,
                        op0=Alu.is_gt,
                    )
                else:
                    nc.scalar.activation(
                        o8[:, T - 1, sl], xt[:, T - 1, sl], AF.Sign, bias=-VTH
                    )
                nc.scalar.dma_start(o_d[:, T - 1, sl], o8[:, T - 1, sl])
    nc.compile()
    return nc


def _get_nc(key=None):
    k = key or CFG
    if k not in _CACHE:
        _CACHE[k] = _build_nc(k)
    return _CACHE[k]


def _shard(x: np.ndarray):
    xs = np.ascontiguousarray(x, dtype=np.float32)
    return [
        np.ascontiguousarray(
            xs[i * B_LOC : (i + 1) * B_LOC].reshape(P, NPP, T).transpose(0, 2, 1)
        )
        for i in range(N_CORES)
    ]


def _run(in_maps, key=None, **kwargs):
    from concourse.bass_utils import run_bass_kernel_spmd

    nc = _get_nc(key)
    return run_bass_kernel_spmd(nc, in_maps, core_ids=list(range(N_CORES)), **kwargs)


def kernel(x: np.ndarray) -> np.ndarray:
    in_maps = [{"x": s} for s in _shard(x)]
    res = _run(in_maps)
    outs = []
    for i in range(N_CORES):
        s8 = res.results[i]["o"]  # [P, T, NPP] int8 sign values
        o = (s8 > 0).transpose(0, 2, 1).astype(np.float32)  # [P, NPP, T]
        outs.append(o.reshape(B_LOC, 128, 32, 32, T))
    return np.concatenate(outs, axis=0)


# revision 21
# speedup vs baseline: 1.1495x; 1.0032x over previous
"""LIF spike recurrence kernel for Trainium2 (8 NeuronCores, SPMD). v12.

Problem: x [32, 128, 32, 32, 8] f32, recurrence over last (time) dim:
    u_t = TAU * u_{t-1} * (1 - o_{t-1}) + x_t
    o_t = 1[u_t - VTH > 0]
Output: o [32, 128, 32, 32, 8] f32 (0.0 / 1.0 spikes).

Design (all facts hardware-probed):
  - Shard batch (32) across 8 cores -> 4/core; host pre-transposes each shard
    to plane-major [P=128, T=8, NPP=4096] so every SBUF access is contiguous
    (strided fp32 STT costs ~1.6x; contiguous runs at (FD+151)/0.96 ns exact,
    back-to-back with ~40ns gaps).
  - One mega-tile [P, T, NPP] per core; input DMA chunked (planes 0/1 in
    interleaved column chunks, so step-1 compute starts ~4 us earlier).
  - Exact fp32 recurrence on DVE, bit-identical to the reference:
       c   = (u_{t-1} <= VTH) * u_{t-1}     (STT is_le/mult; x{0,1} exact)
       u_t = c * TAU + x_t   in place       (TAU=2^-2 exact; single round)
  - Spike via ScalarE: o8_t = Sign(u_t - VTH) -> int8 {-1,0,1}, one ACTIVATE
    per plane ((FD+352)/1.2 ns, no bubble), fully hidden under DVE. Host maps
    >0 to 1.0f (exact). int8 output cuts out-DMA 4x vs f32.
  - Output DMA rides the Scalar HWDGE ring (input uses Sync's) so the queues
    never serialize.
  - v12 = v6 structure with config knobs for input-chunking/ring-splitting
    experiments. A/B-tested on hardware and all kept OFF in the default
    config: finer plane-0/1 interleave (h1024), scalar-ring input assists
    (sr/sp24), and an extra tail split (t8) each measured neutral-to-worse;
    the one-ring stream at the 365 GB/s per-core HBM peak plus the 62us
    serial DVE chain is the measured hardware floor for this recurrence
    (Pool ALU, PE fp32 matmul, and SWDGE accum-DMA offloads all probed
    slower -- see git/session notes).
  - Step 1 is column-chunked and step 7 column-quartered (signs/outs
    interleaved) to shorten the pipeline head and tail.
"""

import numpy as np

TAU = 0.25
VTH = 0.3
N_CORES = 8
P = 128
T = 8
B_LOC = 4  # batches per core
PIX_PER_CORE = B_LOC * 128 * 32 * 32  # 524288
NPP = PIX_PER_CORE // P  # 4096 pixels per partition

_CACHE = {}

# Config key (A/B-tested on hardware):
#   h<k>  : planes 0/1 interleave chunk size k
#   sr<n> : second halves of planes 2..n+1 ride the Scalar ring
#   sp24  : planes 2 and 4 ride the Scalar ring whole (enqueued before any
#           output DMA), pulling early-plane arrivals ahead of the DVE chain
#   t8    : split the last step-7 quarter into two 512-col chunks
CFG = "v12_h2048_sr0"


def _parse(key):
    h, sr = 1024, 3
    for tok in key.split("_"):
        if tok.startswith("h") and tok[1:].isdigit():
            h = int(tok[1:])
        if tok.startswith("sr") and tok[2:].isdigit():
            sr = int(tok[2:])
    acc = 0
    for tok in key.split("_"):
        if tok.startswith("acc") and tok[3:].isdigit():
            acc = int(tok[3:])
    return dict(
        h=h, sr=sr, noaeb="noaeb" in key,
        sp24="sp24" in key, t8="t8" in key, acc=acc,
    )


def _build_nc(key=None):
    if key is None:
        key = CFG
    cfg = _parse(key)
    import concourse.tile as tile
    from concourse import bacc, mybir

    f32 = mybir.dt.float32
    i8 = mybir.dt.int8
    Alu = mybir.AluOpType
    AF = mybir.ActivationFunctionType

    nc = bacc.Bacc(
        "TRN2",
        target_bir_lowering=False,
        debug=False,
        enable_asserts=False,
        num_devices=N_CORES,
    )
    x_d = nc.dram_tensor("x", [P, T, NPP], f32, kind="ExternalInput").ap()
    o_d = nc.dram_tensor("o", [P, T, NPP], i8, kind="ExternalOutput").ap()

    # ACT activation bias needs a pre-registered const AP.
    cb = nc.alloc_sbuf_tensor("const-f32-negvth", [128, 1], f32)
    nc.gpsimd.memset(cb.ap(), -VTH)
    nc.const_aps.aps[(f32, -VTH)] = cb.ap()
    if cfg["noaeb"]:
        nc.multi_engine_barrier(
            [mybir.EngineType.Pool, mybir.EngineType.Activation]
        )
    else:
        nc.all_engine_barrier()

    H = cfg["h"]
    SR = cfg["sr"]  # planes 2..SR+1 second halves on scalar ring

    with tile.TileContext(nc) as tc:
        with tc.tile_pool(name="pp", bufs=1) as pp:
            xt = pp.tile([P, T, NPP], f32, tag="xt")
            c = pp.tile([P, NPP], f32, tag="c")
            o8 = pp.tile([P, T, NPP], i8, tag="o8")

            # Scalar-ring planes first: their enqueues precede every output
            # DMA on that ring, so they stream concurrently with Sync's.
            sp = {2, 4} if cfg["sp24"] else set()
            for t in sorted(sp):
                nc.scalar.dma_start(xt[:, t, :], x_d[:, t, :])
            # Input: planes 0/1 interleaved in H-column chunks on Sync.
            # With acc>0, cols [0,A) of planes 1..6 arrive via the SWDGE
            # accumulate path instead (enqueued next to their STTs below).
            A = cfg["acc"]
            bounds = list(range(0, NPP, H)) + [NPP]
            chunks = list(zip(bounds[:-1], bounds[1:]))
            for lo, hi in chunks:
                nc.sync.dma_start(xt[:, 0, lo:hi], x_d[:, 0, lo:hi])
                l1 = max(lo, A)
                if l1 < hi:
                    nc.sync.dma_start(xt[:, 1, l1:hi], x_d[:, 1, l1:hi])
            # Planes 2..SR+1: first half Sync, second half Scalar ring
            # (enqueued before any output DMA so they clear the ring first).
            for t in range(2, T):
                if t in sp:
                    continue
                a0 = A if t < T - 1 else 0
                if 2 <= t < 2 + SR:
                    nc.sync.dma_start(xt[:, t, a0:2048], x_d[:, t, a0:2048])
                    nc.scalar.dma_start(xt[:, t, 2048:], x_d[:, t, 2048:])
                else:
                    nc.sync.dma_start(xt[:, t, a0:], x_d[:, t, a0:])

            def cu(t, sl):
                up = xt[:, t - 1, sl]
                nc.vector.scalar_tensor_tensor(
                    c[:, sl], up, VTH, up, op0=Alu.is_le, op1=Alu.mult
                )
                nc.vector.scalar_tensor_tensor(
                    xt[:, t, sl], c[:, sl], TAU, xt[:, t, sl],
                    op0=Alu.mult, op1=Alu.add,
                )

            # Accum-chain (cols [0,A) in w-space, w_t = u_t*4^t; host
            # pre-scales those columns so no TAU multiply is needed):
            # DVE writes c_w into the plane-t slot, the SWDGE DMA adds
            # x'_t on top (probed bit-exact fp32).
            def th(j):
                return VTH * (4.0 ** j)

            def stt_acc(t, lo, hi):
                sl = slice(lo, hi)
                up = xt[:, t - 1, sl]
                nc.vector.scalar_tensor_tensor(
                    xt[:, t, sl], up, th(t - 1), up,
                    op0=Alu.is_le, op1=Alu.mult,
                )
                nc.gpsimd.dma_start(
                    xt[:, t, sl], x_d[:, t, sl], accum_op=Alu.add
                )

            def cu_w(t, sl):
                # w-space classic: c = (w<=TH)*w ; w' = c + x' (no TAU).
                up = xt[:, t - 1, sl]
                nc.vector.scalar_tensor_tensor(
                    c[:, sl], up, th(t - 1), up, op0=Alu.is_le, op1=Alu.mult
                )
                nc.vector.tensor_tensor(
                    xt[:, t, sl], c[:, sl], xt[:, t, sl], op=Alu.add
                )

            def signs(t, lo, hi):
                # Mixed-space sign: [0,A) needs scale 4^-t, [A,) scale 1.
                if lo < A and t > 0:
                    nc.scalar.activation(
                        o8[:, t, lo:A], xt[:, t, lo:A], AF.Sign,
                        bias=-VTH, scale=0.25 ** t,
                    )
                    lo = A
                if lo < hi:
                    nc.scalar.activation(
                        o8[:, t, lo:hi], xt[:, t, lo:hi], AF.Sign, bias=-VTH
                    )

            # Plane 0: u_0 = x_0, spike immediately.
            nc.scalar.activation(o8[:, 0, :], xt[:, 0, :], AF.Sign, bias=-VTH)
            nc.scalar.dma_start(o_d[:, 0, :], o8[:, 0, :])

            # Step 1: chunked (chases the chunked DMAs).
            for lo, hi in [(l, m) for l, m in zip([0, A // 2], [A // 2, A]) if A]:
                stt_acc(1, lo, hi)
            for lo, hi in chunks:
                if hi <= A:
                    continue
                cu(1, slice(max(lo, A), hi))
            signs(1, 0, NPP)
            nc.scalar.dma_start(o_d[:, 1, :], o8[:, 1, :])

            # Steps 2..T-2: accum chain first (starts its DMA flights
            # early), then the classic full-width SBUF chain.
            for t in range(2, T - 1):
                if A:
                    stt_acc(t, 0, A // 2)
                    stt_acc(t, A // 2, A)
                cu(t, slice(A, NPP))
                signs(t, 0, NPP)
                nc.scalar.dma_start(o_d[:, t, :], o8[:, t, :])

            # Step T-1: chunked with interleaved signs/outs (short tail); the
            # final chunks spike on the (now otherwise-done) DVE via is_gt ->
            # int8 {0,1} (host maps >0 so both encodings work).
            t7b = sorted({0, A, 1024, 2048, 3072, NPP})
            if cfg["t8"]:
                t7b = sorted({*t7b, 3584})
            t7chunks = list(zip(t7b[:-1], t7b[1:]))
            for q, (lo, hi) in enumerate(t7chunks):
                sl = slice(lo, hi)
                if hi <= A:
                    cu_w(T - 1, sl)
                    signs(T - 1, lo, hi)
                else:
                    cu(T - 1, sl)
                    if q >= len(t7chunks) - (2 if cfg["t8"] else 1):
                        nc.vector.tensor_scalar(
                            o8[:, T - 1, sl], xt[:, T - 1, sl], VTH, None,
                            op0=Alu.is_gt,
                        )
                    else:
                        nc.scalar.activation(
                            o8[:, T - 1, sl], xt[:, T - 1, sl], AF.Sign,
                            bias=-VTH,
                        )
                nc.scalar.dma_start(o_d[:, T - 1, sl], o8[:, T - 1, sl])
    nc.compile()
    return nc


def _get_nc(key=None):
    k = key or CFG
    if k not in _CACHE:
        _CACHE[k] = _build_nc(k)
    return _CACHE[k]


def _shard(x: np.ndarray, key=None):
    A = _parse(key or CFG)["acc"]
    xs = np.ascontiguousarray(x, dtype=np.float32)
    out = []
    for i in range(N_CORES):
        s = np.ascontiguousarray(
            xs[i * B_LOC : (i + 1) * B_LOC].reshape(P, NPP, T).transpose(0, 2, 1)
        )
        if A:
            # w-space for the accum columns: plane t scaled by 4^t (exact).
            for t in range(1, T):
                s[:, t, :A] *= np.float32(4.0 ** t)
        out.append(s)
    return out


def _run(in_maps, key=None, **kwargs):
    from concourse.bass_utils import run_bass_kernel_spmd

    nc = _get_nc(key)
    return run_bass_kernel_spmd(nc, in_maps, core_ids=list(range(N_CORES)), **kwargs)


def kernel(x: np.ndarray) -> np.ndarray:
    in_maps = [{"x": s} for s in _shard(x)]
    res = _run(in_maps)
    outs = []
    for i in range(N_CORES):
        s8 = res.results[i]["o"]  # [P, T, NPP] int8 sign values
        o = (s8 > 0).transpose(0, 2, 1).astype(np.float32)  # [P, NPP, T]
        outs.append(o.reshape(B_LOC, 128, 32, 32, T))
    return np.concatenate(outs, axis=0)
